# revision 2
# baseline (speedup 1.0000x reference)
"""Trainium2 Bass kernel for ChronoRotationTransformation.

Computes, per batch row b (B=8192, D=2048):
    u   = (head_r + i*head_i) * (rel_r + i*rel_i)          # complex product
    ab  = sum_d u_r*tail_r - u_i*tail_i                    # == sum rot_r*t_r + rot_i*t_i
    aa  = sum_d u_r^2 + u_i^2                              # == |rot|^2
    bb  = sum_d tail_r^2 + tail_i^2
    out = ab / sqrt(aa*bb)

(The reference's rot = conj(head*rel); rot_r = u_r, rot_i = -u_i, so
ab = rot_r*t_r + rot_i*t_i = u_r*t_r - u_i*t_i and |rot|^2 = |u|^2.)

Sharding: pure data-parallel across 8 NeuronCores, 1024 rows each.
Per core: 8 row-tiles of [128, 2048]. DVE does the 4 cross products,
the two add/subs forming u, and two fused multiply+reduce (ab); ACT
does 4 square+accumulate reductions (aa, bb). Memory-bound target:
~48 MiB HBM reads per core.
"""

import numpy as np

B, D = 8192, 2048
NCORES = 8
BC = B // NCORES            # rows per core
P = 128                     # SBUF partitions
NT = BC // P                # row-tiles per core

IN_NAMES = [
    "head_real", "head_imag",
    "rel_real", "rel_imag",
    "tail_real", "tail_imag",
]

_CACHE = {}


def _emit_v10(tc, ins, out_ap, mybir, repeats=1):
    """Wide-tile variant: pairs (hr|hi), (rr|ri), (tr|ti) live in [P, 2D]
    tiles. The 4 products collapse into 2 wide DVE muls via stride-0
    broadcast APs of rr / ri, and the 4 ACT square-reduces into 2 wide
    ones. DVE: 6 insts/tile (was 8) over the same 16K columns — trims
    per-instruction overhead off the DVE-bound critical path."""
    nc = tc.nc
    f32 = mybir.dt.float32
    Alu = mybir.AluOpType
    Act = mybir.ActivationFunctionType
    D2 = 2 * D

    dv = {n: ins[n].rearrange("(t p) d -> t p d", p=P) for n in IN_NAMES}
    out_d = out_ap.rearrange("(t p) -> p t", p=P)

    with (
        tc.tile_pool(name="inp", bufs=2) as inp,
        tc.tile_pool(name="prod", bufs=1) as prod,
        tc.tile_pool(name="upool", bufs=2) as upool,
        tc.tile_pool(name="scr", bufs=1) as scr,
        tc.tile_pool(name="stats", bufs=1) as stats,
    ):
        ab1_s = stats.tile([P, NT], f32, tag="ab1_s")
        ab2_s = stats.tile([P, NT], f32, tag="ab2_s")
        aa_s = stats.tile([P, NT], f32, tag="aa_s")
        bb_s = stats.tile([P, NT], f32, tag="bb_s")

        for _rep in range(repeats):
          for t in range(NT):
            H = inp.tile([P, D2], f32, tag="H")
            nc.sync.dma_start(out=H[:, 0:D], in_=dv["head_real"][t])
            nc.sync.dma_start(out=H[:, D:D2], in_=dv["head_imag"][t])
            R = inp.tile([P, D2], f32, tag="R")
            nc.sync.dma_start(out=R[:, 0:D], in_=dv["rel_real"][t])
            nc.sync.dma_start(out=R[:, D:D2], in_=dv["rel_imag"][t])
            T = inp.tile([P, D2], f32, tag="T", bufs=3)
            nc.sync.dma_start(out=T[:, 0:D], in_=dv["tail_real"][t])
            nc.sync.dma_start(out=T[:, D:D2], in_=dv["tail_imag"][t])

            rr_b = (R[:, 0:D].rearrange("p (one d) -> p one d", one=1)
                    .broadcast_to([P, 2, D]))
            ri_b = (R[:, D:D2].rearrange("p (one d) -> p one d", one=1)
                    .broadcast_to([P, 2, D]))

            # A = m1|m3 = hr*rr | hi*rr ; B = m4|m2 = hr*ri | hi*ri
            A = prod.tile([P, D2], f32, tag="A")
            nc.vector.tensor_mul(A[:], H[:], rr_b)
            Bt = prod.tile([P, D2], f32, tag="B")
            nc.vector.tensor_mul(Bt[:], H[:], ri_b)

            # ur = m1 - m2 = A_lo - B_hi ; ui = m3 + m4 = A_hi + B_lo
            U = upool.tile([P, D2], f32, tag="U")
            nc.vector.tensor_sub(U[:, 0:D], A[:, 0:D], Bt[:, D:D2])
            nc.vector.tensor_add(U[:, D:D2], A[:, D:D2], Bt[:, 0:D])

            # ab = sum(ur*tr) - sum(ui*ti); scratch outs alias dead A/B.
            so1 = prod.tile([P, D2], f32, tag="A")
            nc.vector.scalar_tensor_tensor(
                out=so1[:, 0:D], in0=U[:, 0:D], scalar=1.0, in1=T[:, 0:D],
                op0=Alu.mult, op1=Alu.mult, accum_out=ab1_s[:, t:t + 1],
            )
            so2 = prod.tile([P, D2], f32, tag="B")
            nc.vector.scalar_tensor_tensor(
                out=so2[:, 0:D], in0=U[:, D:D2], scalar=-1.0, in1=T[:, D:D2],
                op0=Alu.mult, op1=Alu.mult, accum_out=ab2_s[:, t:t + 1],
            )

            # bb = sum(tr^2 + ti^2), aa = sum(ur^2 + ui^2): one wide
            # square-accumulate each on ACT.
            ao1 = scr.tile([P, D2], f32, tag="ao")
            nc.scalar.activation(
                out=ao1[:], in_=T[:], func=Act.Square,
                accum_out=bb_s[:, t:t + 1],
            )
            ao2 = scr.tile([P, D2], f32, tag="ao")
            nc.scalar.activation(
                out=ao2[:], in_=U[:], func=Act.Square,
                accum_out=aa_s[:, t:t + 1],
            )

        # Final combine on [P, NT] (tiny).
        fin = {}
        def ftile(name):
            tl = stats.tile([P, NT], f32, tag=name)
            fin[name] = tl
            return tl

        ab = ftile("ab"); nc.vector.tensor_add(ab[:], ab1_s[:], ab2_s[:])
        pp = ftile("pp"); nc.vector.tensor_mul(pp[:], aa_s[:], bb_s[:])
        r = ftile("r0"); nc.scalar.activation(out=r[:], in_=pp[:], func=Act.Sqrt)
        for it in range(2):
            q = ftile(f"q{it}"); nc.vector.reciprocal(q[:], r[:])
            pq = ftile(f"pq{it}"); nc.vector.tensor_mul(pq[:], pp[:], q[:])
            s = ftile(f"s{it}"); nc.vector.tensor_add(s[:], r[:], pq[:])
            r = ftile(f"r{it + 1}"); nc.vector.tensor_scalar_mul(r[:], s[:], 0.5)
        inv = ftile("inv"); nc.vector.reciprocal(inv[:], r[:])
        score = ftile("score"); nc.vector.tensor_mul(score[:], ab[:], inv[:])
        nc.sync.dma_start(out=out_d, in_=score[:])


def _emit(tc, ins, out_ap, mybir, repeats=1, cfg="v4"):
    import concourse.bass as bass  # noqa: F401

    if cfg == "v10":
        return _emit_v10(tc, ins, out_ap, mybir, repeats=repeats)

    nc = tc.nc
    f32 = mybir.dt.float32
    Alu = mybir.AluOpType
    Act = mybir.ActivationFunctionType

    # DRAM views: [NT, P, D] row-tiles; out as [P, NT] (row = t*128 + p).
    dv = {n: ins[n].rearrange("(t p) d -> t p d", p=P) for n in IN_NAMES}
    out_d = out_ap.rearrange("(t p) -> p t", p=P)

    with (
        tc.tile_pool(name="inp", bufs=2) as inp,
        tc.tile_pool(name="prod", bufs=1) as prod,
        tc.tile_pool(name="upool", bufs=2) as upool,
        tc.tile_pool(name="scr", bufs=1) as scr,
        tc.tile_pool(name="stats", bufs=1) as stats,
    ):
        ab1_s = stats.tile([P, NT], f32, tag="ab1_s")
        ab2_s = stats.tile([P, NT], f32, tag="ab2_s")
        aa1_s = stats.tile([P, NT], f32, tag="aa1_s")
        aa2_s = stats.tile([P, NT], f32, tag="aa2_s")
        bb1_s = stats.tile([P, NT], f32, tag="bb1_s")
        bb2_s = stats.tile([P, NT], f32, tag="bb2_s")

        for _rep in range(repeats):
          for t in range(NT):
            tiles = {}
            # v8/v9: the Pool-offloaded products read head_imag/rel_real
            # (/rel_imag) — DMA those first so Pool starts a full tile
            # ahead and DVE's ui add never waits on it.
            load_order = (
                ["head_imag", "rel_real", "head_real", "rel_imag",
                 "tail_real", "tail_imag"] if cfg in ("v8", "v9") else IN_NAMES
            )
            for n in load_order:
                # tail tiles are the last-released each tile (read by the
                # STT dots at the end) — give them one extra buffer so
                # their next DMA isn't gated on the ring.
                if cfg in ("v8", "v9"):
                    # SBUF budget ~208KB/partition: 3-deep only for the
                    # Pool-read tiles, 2 for everything else.
                    nb = 3 if n in ("head_imag", "rel_real") else 2
                else:
                    nb = 3 if (cfg == "v5" or n.startswith("tail")) else 2
                tl = inp.tile([P, D], f32, tag=n, bufs=nb)
                nc.sync.dma_start(out=tl[:], in_=dv[n][t])
                tiles[n] = tl
            hr, hi = tiles["head_real"], tiles["head_imag"]
            rr, ri = tiles["rel_real"], tiles["rel_imag"]
            tr, ti = tiles["tail_real"], tiles["tail_imag"]

            # The HW-measured compute floor is DVE-bound (8 f32 [P,D]
            # passes/tile at ~2.26us; f32 tensor_tensor does NOT get the
            # cost model's 2x_1p speedup on HW). Pool (GPSIMD) is idle, so
            # v6/v7 offload 1 / 1.5 of the passes to it, taking DVE to
            # 7 / 6.5 passes — toward the ~123us DMA floor.
            m3 = prod.tile([P, D], f32, tag="m3",
                           bufs=2 if cfg in ("v6", "v7", "v8", "v9") else 1)
            if cfg in ("v6", "v7", "v8", "v9"):
                nc.gpsimd.tensor_mul(m3[:], hi[:], rr[:])
            else:
                nc.vector.tensor_mul(m3[:], hi[:], rr[:])
            m4 = prod.tile([P, D], f32, tag="m4",
                           bufs=2 if cfg == "v9" else 1)
            if cfg == "v9":
                nc.gpsimd.tensor_mul(m4[:], hr[:], ri[:])
            else:
                nc.vector.tensor_mul(m4[:], hr[:], ri[:])
            m1 = prod.tile([P, D], f32, tag="m1")
            nc.vector.tensor_mul(m1[:], hr[:], rr[:])
            m2 = prod.tile([P, D], f32, tag="m2")
            nc.vector.tensor_mul(m2[:], hi[:], ri[:])
            ub = 1 if cfg == "v5" else 2
            ur = upool.tile([P, D], f32, tag="ur", bufs=ub)
            nc.vector.tensor_sub(ur[:], m1[:], m2[:])
            ui = upool.tile([P, D], f32, tag="ui", bufs=ub)
            if cfg == "v7" and t % 2 == 0:
                nc.gpsimd.tensor_add(ui[:], m3[:], m4[:])
            else:
                nc.vector.tensor_add(ui[:], m3[:], m4[:])

            # ab = sum(ur*tr) - sum(ui*ti): fused multiply+reduce via
            # scalar_tensor_tensor (out = (in0 op0 scalar) op1 in1,
            # accum_out = sum(out)). tensor_tensor_reduce (native TTR
            # opcode) crashes this terminal's NRT — do not use it.
            # Scratch outs alias the dead m1/m2 slots (same pool tag) —
            # WAR/WAW stay on-engine, zero extra SBUF.
            so1 = prod.tile([P, D], f32, tag="m1")
            nc.vector.scalar_tensor_tensor(
                out=so1[:], in0=ur[:], scalar=1.0, in1=tr[:],
                op0=Alu.mult, op1=Alu.mult, accum_out=ab1_s[:, t:t + 1],
            )
            so2 = prod.tile([P, D], f32, tag="m2")
            nc.vector.scalar_tensor_tensor(
                out=so2[:], in0=ui[:], scalar=-1.0, in1=ti[:],
                op0=Alu.mult, op1=Alu.mult, accum_out=ab2_s[:, t:t + 1],
            )

            # aa, bb: square+accumulate on ACT. bb first — tr/ti are
            # already resident before DVE finishes the products, so ACT
            # starts early and tr/ti stay hot for the STT dots.
            for src, dst in (
                (tr, bb1_s), (ti, bb2_s), (ur, aa1_s), (ui, aa2_s),
            ):
                ao = scr.tile([P, D], f32, tag="ao")
                nc.scalar.activation(
                    out=ao[:], in_=src[:], func=Act.Square,
                    accum_out=dst[:, t:t + 1],
                )

        # Final combine on [P, NT] (tiny).
        fin = {}
        def ftile(name):
            tl = stats.tile([P, NT], f32, tag=name)
            fin[name] = tl
            return tl

        ab = ftile("ab"); nc.vector.tensor_add(ab[:], ab1_s[:], ab2_s[:])
        aa = ftile("aa"); nc.vector.tensor_add(aa[:], aa1_s[:], aa2_s[:])
        bb = ftile("bb"); nc.vector.tensor_add(bb[:], bb1_s[:], bb2_s[:])
        pp = ftile("pp"); nc.vector.tensor_mul(pp[:], aa[:], bb[:])
        # sqrt on ACT is low precision (up to ~65536 ULP budget); refine
        # with two Newton iterations  r <- 0.5*(r + p/r)  using the
        # bit-exact DVE reciprocal.
        r = ftile("r0"); nc.scalar.activation(out=r[:], in_=pp[:], func=Act.Sqrt)
        for it in range(2):
            q = ftile(f"q{it}"); nc.vector.reciprocal(q[:], r[:])
            pq = ftile(f"pq{it}"); nc.vector.tensor_mul(pq[:], pp[:], q[:])
            s = ftile(f"s{it}"); nc.vector.tensor_add(s[:], r[:], pq[:])
            r = ftile(f"r{it + 1}"); nc.vector.tensor_scalar_mul(r[:], s[:], 0.5)
        inv = ftile("inv"); nc.vector.reciprocal(inv[:], r[:])
        score = ftile("score"); nc.vector.tensor_mul(score[:], ab[:], inv[:])
        nc.sync.dma_start(out=out_d, in_=score[:])


def _build(repeats=1, cfg="v10"):
    key = ("nc", repeats, cfg)
    if key in _CACHE:
        return _CACHE[key]
    import concourse.tile as tile
    from concourse import bacc, mybir

    # NOTE: num_devices is deliberately NOT set — it enables collective
    # global-comm setup that breaks plain SPMD input binding under the
    # axon/PJRT path (outputs come back as garbage).
    nc = bacc.Bacc(
        "TRN2",
        target_bir_lowering=False,
        debug=False,
    )
    ins = {
        n: nc.dram_tensor(n, [BC, D], mybir.dt.float32, kind="ExternalInput").ap()
        for n in IN_NAMES
    }
    out = nc.dram_tensor("out", [BC], mybir.dt.float32, kind="ExternalOutput").ap()
    with tile.TileContext(nc) as tc:
        _emit(tc, ins, out, mybir, repeats=repeats, cfg=cfg)
    nc.compile()
    _CACHE[key] = nc
    return nc


def make_in_maps(inputs):
    """Per-core input dicts: batch-sharded slices, dtype per kernel build."""
    in_maps = []
    for c in range(NCORES):
        sl = slice(c * BC, (c + 1) * BC)
        in_maps.append(
            {n: np.ascontiguousarray(inputs[n][sl], dtype=np.float32)
             for n in IN_NAMES}
        )
    return in_maps


def run(inputs, trace=False, **kwargs):
    """Run on 8 cores; returns (full_output, BassKernelResults)."""
    from concourse.bass_utils import run_bass_kernel_spmd

    nc = _build()
    core_ids = list(range(NCORES))
    in_maps = make_in_maps(inputs)
    # The terminal occasionally reports the accelerator unrecoverable
    # (e.g. poisoned by an earlier crashed run); a fresh attempt after a
    # short wait triggers recovery.
    last_exc = None
    for attempt in range(4):
        try:
            res = run_bass_kernel_spmd(nc, in_maps, core_ids, trace=trace, **kwargs)
            break
        except Exception as e:  # noqa: BLE001
            last_exc = e
            if attempt == 3:
                raise
            import time as _time
            _time.sleep(15 * (attempt + 1))
    out = np.concatenate([res.results[c]["out"] for c in range(NCORES)])
    return out.astype(np.float32), res


def kernel(**inputs):
    out, _ = run(inputs)
    return out



# revision 9
# speedup vs baseline: 1.0368x; 1.0368x over previous
"""Trainium2 Bass kernel for ChronoRotationTransformation.

Computes, per batch row b (B=8192, D=2048):
    u   = (head_r + i*head_i) * (rel_r + i*rel_i)          # complex product
    ab  = sum_d u_r*tail_r - u_i*tail_i                    # == sum rot_r*t_r + rot_i*t_i
    aa  = sum_d u_r^2 + u_i^2                              # == |rot|^2
    bb  = sum_d tail_r^2 + tail_i^2
    out = ab / sqrt(aa*bb)

(The reference's rot = conj(head*rel); rot_r = u_r, rot_i = -u_i, so
ab = rot_r*t_r + rot_i*t_i = u_r*t_r - u_i*t_i and |rot|^2 = |u|^2.)

Sharding: pure data-parallel across 8 NeuronCores, 1024 rows each.
Per core: 8 row-tiles of [128, 2048]. DVE does the 4 cross products,
the two add/subs forming u, and two fused multiply+reduce (ab); ACT
does 4 square+accumulate reductions (aa, bb). Memory-bound target:
~48 MiB HBM reads per core.
"""

import numpy as np

B, D = 8192, 2048
NCORES = 8
BC = B // NCORES            # rows per core
P = 128                     # SBUF partitions
NT = BC // P                # row-tiles per core

IN_NAMES = [
    "head_real", "head_imag",
    "rel_real", "rel_imag",
    "tail_real", "tail_imag",
]

_CACHE = {}


def _emit_v10(tc, ins, out_ap, mybir, repeats=1):
    """Wide-tile variant: pairs (hr|hi), (rr|ri), (tr|ti) live in [P, 2D]
    tiles. The 4 products collapse into 2 wide DVE muls via stride-0
    broadcast APs of rr / ri, and the 4 ACT square-reduces into 2 wide
    ones. DVE: 6 insts/tile (was 8) over the same 16K columns — trims
    per-instruction overhead off the DVE-bound critical path."""
    nc = tc.nc
    f32 = mybir.dt.float32
    Alu = mybir.AluOpType
    Act = mybir.ActivationFunctionType
    D2 = 2 * D

    dv = {n: ins[n].rearrange("(t p) d -> t p d", p=P) for n in IN_NAMES}
    out_d = out_ap.rearrange("(t p) -> p t", p=P)

    with (
        tc.tile_pool(name="inp", bufs=2) as inp,
        tc.tile_pool(name="prod", bufs=1) as prod,
        tc.tile_pool(name="upool", bufs=2) as upool,
        tc.tile_pool(name="scr", bufs=1) as scr,
        tc.tile_pool(name="stats", bufs=1) as stats,
    ):
        ab1_s = stats.tile([P, NT], f32, tag="ab1_s")
        ab2_s = stats.tile([P, NT], f32, tag="ab2_s")
        aa_s = stats.tile([P, NT], f32, tag="aa_s")
        bb_s = stats.tile([P, NT], f32, tag="bb_s")

        for _rep in range(repeats):
          for t in range(NT):
            H = inp.tile([P, D2], f32, tag="H")
            nc.sync.dma_start(out=H[:, 0:D], in_=dv["head_real"][t])
            nc.sync.dma_start(out=H[:, D:D2], in_=dv["head_imag"][t])
            R = inp.tile([P, D2], f32, tag="R")
            nc.sync.dma_start(out=R[:, 0:D], in_=dv["rel_real"][t])
            nc.sync.dma_start(out=R[:, D:D2], in_=dv["rel_imag"][t])
            T = inp.tile([P, D2], f32, tag="T", bufs=3)
            nc.sync.dma_start(out=T[:, 0:D], in_=dv["tail_real"][t])
            nc.sync.dma_start(out=T[:, D:D2], in_=dv["tail_imag"][t])

            rr_b = (R[:, 0:D].rearrange("p (one d) -> p one d", one=1)
                    .broadcast_to([P, 2, D]))
            ri_b = (R[:, D:D2].rearrange("p (one d) -> p one d", one=1)
                    .broadcast_to([P, 2, D]))

            # A = m1|m3 = hr*rr | hi*rr ; B = m4|m2 = hr*ri | hi*ri
            A = prod.tile([P, D2], f32, tag="A")
            nc.vector.tensor_mul(A[:], H[:], rr_b)
            Bt = prod.tile([P, D2], f32, tag="B")
            nc.vector.tensor_mul(Bt[:], H[:], ri_b)

            # ur = m1 - m2 = A_lo - B_hi ; ui = m3 + m4 = A_hi + B_lo
            U = upool.tile([P, D2], f32, tag="U")
            nc.vector.tensor_sub(U[:, 0:D], A[:, 0:D], Bt[:, D:D2])
            nc.vector.tensor_add(U[:, D:D2], A[:, D:D2], Bt[:, 0:D])

            # ab = sum(ur*tr) - sum(ui*ti); scratch outs alias dead A/B.
            so1 = prod.tile([P, D2], f32, tag="A")
            nc.vector.scalar_tensor_tensor(
                out=so1[:, 0:D], in0=U[:, 0:D], scalar=1.0, in1=T[:, 0:D],
                op0=Alu.mult, op1=Alu.mult, accum_out=ab1_s[:, t:t + 1],
            )
            so2 = prod.tile([P, D2], f32, tag="B")
            nc.vector.scalar_tensor_tensor(
                out=so2[:, 0:D], in0=U[:, D:D2], scalar=-1.0, in1=T[:, D:D2],
                op0=Alu.mult, op1=Alu.mult, accum_out=ab2_s[:, t:t + 1],
            )

            # bb = sum(tr^2 + ti^2), aa = sum(ur^2 + ui^2): one wide
            # square-accumulate each on ACT.
            ao1 = scr.tile([P, D2], f32, tag="ao")
            nc.scalar.activation(
                out=ao1[:], in_=T[:], func=Act.Square,
                accum_out=bb_s[:, t:t + 1],
            )
            ao2 = scr.tile([P, D2], f32, tag="ao")
            nc.scalar.activation(
                out=ao2[:], in_=U[:], func=Act.Square,
                accum_out=aa_s[:, t:t + 1],
            )

        # Final combine on [P, NT] (tiny).
        fin = {}
        def ftile(name):
            tl = stats.tile([P, NT], f32, tag=name)
            fin[name] = tl
            return tl

        ab = ftile("ab"); nc.vector.tensor_add(ab[:], ab1_s[:], ab2_s[:])
        pp = ftile("pp"); nc.vector.tensor_mul(pp[:], aa_s[:], bb_s[:])
        r = ftile("r0"); nc.scalar.activation(out=r[:], in_=pp[:], func=Act.Sqrt)
        for it in range(2):
            q = ftile(f"q{it}"); nc.vector.reciprocal(q[:], r[:])
            pq = ftile(f"pq{it}"); nc.vector.tensor_mul(pq[:], pp[:], q[:])
            s = ftile(f"s{it}"); nc.vector.tensor_add(s[:], r[:], pq[:])
            r = ftile(f"r{it + 1}"); nc.vector.tensor_scalar_mul(r[:], s[:], 0.5)
        inv = ftile("inv"); nc.vector.reciprocal(inv[:], r[:])
        score = ftile("score"); nc.vector.tensor_mul(score[:], ab[:], inv[:])
        nc.sync.dma_start(out=out_d, in_=score[:])


def _emit_dma(tc, ins, out_ap, mybir, repeats=1, cfg="dma"):
    """DMA-only floor probes: stream all 48 MiB, no per-element compute.
    Output is garbage (zeros) — bench-only, never correctness-checked.
    dma: 6x [P,D] loads per tile on the sync HWDGE ring.
    dma2: same, alternating sync/scalar HWDGE rings.
    dmaw: 2 row-tiles per DMA ([P,2D] from [2,P,D] DRAM view), half the count."""
    nc = tc.nc
    f32 = mybir.dt.float32
    dv = {n: ins[n].rearrange("(t p) d -> t p d", p=P) for n in IN_NAMES}
    out_d = out_ap.rearrange("(t p) -> p t", p=P)

    with (
        tc.tile_pool(name="inp", bufs=2) as inp,
        tc.tile_pool(name="stats", bufs=1) as stats,
    ):
        for _rep in range(repeats):
            if cfg == "dmaw":
                for t in range(0, NT, 2):
                    for n in IN_NAMES:
                        tl = inp.tile([P, 2 * D], f32, tag=n)
                        src = dv[n][t:t + 2].rearrange("two p d -> p two d")
                        dst = tl[:].rearrange("p (two d) -> p two d", two=2)
                        nc.sync.dma_start(out=dst, in_=src)
            else:
                for t in range(NT):
                    for i, n in enumerate(IN_NAMES):
                        tl = inp.tile([P, D], f32, tag=n)
                        eng = nc.scalar if (cfg == "dma2" and i % 2) else nc.sync
                        eng.dma_start(out=tl[:], in_=dv[n][t])
        score = stats.tile([P, NT], f32, tag="score")
        nc.scalar.memzero(score[:])
        nc.sync.dma_start(out=out_d, in_=score[:])


def _emit_v11(tc, ins, out_ap, mybir, repeats=1):
    """4-wide-DVE schedule without materializing u.

    With m1=hr*rr, m2=hi*ri, m3=hi*rr, m4=hr*ri:
      ab = sum m1*tr - m2*tr - m3*ti - m4*ti
      aa = sum (m1-m2)^2 + (m3+m4)^2 = sum m1^2+m2^2+m3^2+m4^2
           (cross terms cancel exactly: m1*m2 == m3*m4)
    DVE per tile: A = H*rr_b (m1|m3), B = H*(-ri)_b (-m4|-m2), and two
    wide dot-STTs against overlapping windows of MEGA = (ti | tr | -ti):
      dotA = A . (tr|-ti) = m1*tr - m3*ti      [MEGA cols D:3D]
      dotB = B . (ti|tr)  = -m4*ti - m2*tr     [MEGA cols 0:2D]
    ACT: Square(A), Square(B) -> aa halves; Square(MEGA[0:2D]) -> bb;
    plus 2 negate fixups (-ri, -ti). Tail: ab = dotA+dotB, aa = sq_A+sq_B,
    score = ab / sqrt(aa*bb) with 1 Newton step on the ACT sqrt."""
    nc = tc.nc
    f32 = mybir.dt.float32
    Alu = mybir.AluOpType
    Act = mybir.ActivationFunctionType
    D2, D3 = 2 * D, 3 * D

    dv = {n: ins[n].rearrange("(t p) d -> t p d", p=P) for n in IN_NAMES}
    out_d = out_ap.rearrange("(t p) -> p t", p=P)

    with (
        tc.tile_pool(name="inp", bufs=2) as inp,
        tc.tile_pool(name="prod", bufs=1) as prod,
        tc.tile_pool(name="scr", bufs=1) as scr,
        tc.tile_pool(name="stats", bufs=1) as stats,
    ):
        ab1_s = stats.tile([P, NT], f32, tag="ab1_s")
        ab2_s = stats.tile([P, NT], f32, tag="ab2_s")
        aa1_s = stats.tile([P, NT], f32, tag="aa1_s")
        aa2_s = stats.tile([P, NT], f32, tag="aa2_s")
        bb_s = stats.tile([P, NT], f32, tag="bb_s")

        for _rep in range(repeats):
          for t in range(NT):
            H = inp.tile([P, D2], f32, tag="H")
            nc.sync.dma_start(out=H[:, 0:D], in_=dv["head_real"][t])
            nc.sync.dma_start(out=H[:, D:D2], in_=dv["head_imag"][t])
            RR = inp.tile([P, D], f32, tag="RR")
            nc.sync.dma_start(out=RR[:], in_=dv["rel_real"][t])
            RI = inp.tile([P, D], f32, tag="RI")
            nc.sync.dma_start(out=RI[:], in_=dv["rel_imag"][t])
            M = inp.tile([P, D3], f32, tag="M")
            nc.sync.dma_start(out=M[:, 0:D], in_=dv["tail_imag"][t])
            nc.sync.dma_start(out=M[:, D:D2], in_=dv["tail_real"][t])

            # ACT fixups: NRI = -ri; MEGA hi third = -ti.
            NRI = inp.tile([P, D], f32, tag="NRI")
            nc.scalar.mul(NRI[:], RI[:], -1.0)
            nc.scalar.mul(M[:, D2:D3], M[:, 0:D], -1.0)

            rr_b = (RR[:].rearrange("p (one d) -> p one d", one=1)
                    .broadcast_to([P, 2, D]))
            nri_b = (NRI[:].rearrange("p (one d) -> p one d", one=1)
                     .broadcast_to([P, 2, D]))

            A = prod.tile([P, D2], f32, tag="A")
            nc.vector.tensor_mul(A[:], H[:], rr_b)       # (m1 | m3)
            Bt = prod.tile([P, D2], f32, tag="B")
            nc.vector.tensor_mul(Bt[:], H[:], nri_b)     # (-m4 | -m2)

            # bb first so ACT consumes MEGA early; aa squares after muls.
            ao1 = scr.tile([P, D2], f32, tag="ao")
            nc.scalar.activation(
                out=ao1[:], in_=M[:, 0:D2],
                func=Act.Square, accum_out=bb_s[:, t:t + 1],
            )

            so1 = scr.tile([P, D2], f32, tag="so")
            nc.vector.scalar_tensor_tensor(
                out=so1[:], in0=Bt[:], scalar=1.0, in1=M[:, 0:D2],
                op0=Alu.mult, op1=Alu.mult, accum_out=ab2_s[:, t:t + 1],
            )
            so2 = scr.tile([P, D2], f32, tag="so")
            nc.vector.scalar_tensor_tensor(
                out=so2[:], in0=A[:], scalar=1.0, in1=M[:, D:D3],
                op0=Alu.mult, op1=Alu.mult, accum_out=ab1_s[:, t:t + 1],
            )

            ao2 = scr.tile([P, D2], f32, tag="ao")
            nc.scalar.activation(
                out=ao2[:], in_=A[:],
                func=Act.Square, accum_out=aa1_s[:, t:t + 1],
            )
            ao3 = scr.tile([P, D2], f32, tag="ao")
            nc.scalar.activation(
                out=ao3[:], in_=Bt[:],
                func=Act.Square, accum_out=aa2_s[:, t:t + 1],
            )

        # Final combine on [P, NT] (tiny).
        fin = {}
        def ftile(name):
            tl = stats.tile([P, NT], f32, tag=name)
            fin[name] = tl
            return tl

        ab = ftile("ab"); nc.vector.tensor_add(ab[:], ab1_s[:], ab2_s[:])
        aa = ftile("aa"); nc.vector.tensor_add(aa[:], aa1_s[:], aa2_s[:])
        pp = ftile("pp"); nc.vector.tensor_mul(pp[:], aa[:], bb_s[:])
        # ACT sqrt is low precision (~65536 ULP budget = ~4e-3 rel); one
        # Newton step r <- 0.5*(r + p/r) brings it to ~8e-6 rel.
        r = ftile("r0"); nc.scalar.activation(out=r[:], in_=pp[:], func=Act.Sqrt)
        q = ftile("q"); nc.vector.reciprocal(q[:], r[:])
        pq = ftile("pq"); nc.vector.tensor_mul(pq[:], pp[:], q[:])
        s = ftile("s"); nc.vector.tensor_add(s[:], r[:], pq[:])
        r1 = ftile("r1"); nc.vector.tensor_scalar_mul(r1[:], s[:], 0.5)
        inv = ftile("inv"); nc.vector.reciprocal(inv[:], r1[:])
        score = ftile("score"); nc.vector.tensor_mul(score[:], ab[:], inv[:])
        nc.sync.dma_start(out=out_d, in_=score[:])


def _emit(tc, ins, out_ap, mybir, repeats=1, cfg="v4"):
    import concourse.bass as bass  # noqa: F401

    if cfg == "v10":
        return _emit_v10(tc, ins, out_ap, mybir, repeats=repeats)
    if cfg == "v11":
        return _emit_v11(tc, ins, out_ap, mybir, repeats=repeats)
    if cfg in ("dma", "dma2", "dmaw"):
        return _emit_dma(tc, ins, out_ap, mybir, repeats=repeats, cfg=cfg)

    nc = tc.nc
    f32 = mybir.dt.float32
    Alu = mybir.AluOpType
    Act = mybir.ActivationFunctionType

    # DRAM views: [NT, P, D] row-tiles; out as [P, NT] (row = t*128 + p).
    dv = {n: ins[n].rearrange("(t p) d -> t p d", p=P) for n in IN_NAMES}
    out_d = out_ap.rearrange("(t p) -> p t", p=P)

    with (
        tc.tile_pool(name="inp", bufs=2) as inp,
        tc.tile_pool(name="prod", bufs=1) as prod,
        tc.tile_pool(name="upool", bufs=2) as upool,
        tc.tile_pool(name="scr", bufs=1) as scr,
        tc.tile_pool(name="stats", bufs=1) as stats,
    ):
        ab1_s = stats.tile([P, NT], f32, tag="ab1_s")
        ab2_s = stats.tile([P, NT], f32, tag="ab2_s")
        aa1_s = stats.tile([P, NT], f32, tag="aa1_s")
        aa2_s = stats.tile([P, NT], f32, tag="aa2_s")
        bb1_s = stats.tile([P, NT], f32, tag="bb1_s")
        bb2_s = stats.tile([P, NT], f32, tag="bb2_s")

        for _rep in range(repeats):
          for t in range(NT):
            tiles = {}
            # v8/v9: the Pool-offloaded products read head_imag/rel_real
            # (/rel_imag) — DMA those first so Pool starts a full tile
            # ahead and DVE's ui add never waits on it.
            load_order = (
                ["head_imag", "rel_real", "head_real", "rel_imag",
                 "tail_real", "tail_imag"] if cfg in ("v8", "v9") else IN_NAMES
            )
            for n in load_order:
                # tail tiles are the last-released each tile (read by the
                # STT dots at the end) — give them one extra buffer so
                # their next DMA isn't gated on the ring.
                if cfg in ("v8", "v9"):
                    # SBUF budget ~208KB/partition: 3-deep only for the
                    # Pool-read tiles, 2 for everything else.
                    nb = 3 if n in ("head_imag", "rel_real") else 2
                else:
                    nb = 3 if (cfg == "v5" or n.startswith("tail")) else 2
                tl = inp.tile([P, D], f32, tag=n, bufs=nb)
                nc.sync.dma_start(out=tl[:], in_=dv[n][t])
                tiles[n] = tl
            hr, hi = tiles["head_real"], tiles["head_imag"]
            rr, ri = tiles["rel_real"], tiles["rel_imag"]
            tr, ti = tiles["tail_real"], tiles["tail_imag"]

            # The HW-measured compute floor is DVE-bound (8 f32 [P,D]
            # passes/tile at ~2.26us; f32 tensor_tensor does NOT get the
            # cost model's 2x_1p speedup on HW). Pool (GPSIMD) is idle, so
            # v6/v7 offload 1 / 1.5 of the passes to it, taking DVE to
            # 7 / 6.5 passes — toward the ~123us DMA floor.
            m3 = prod.tile([P, D], f32, tag="m3",
                           bufs=2 if cfg in ("v6", "v7", "v8", "v9") else 1)
            if cfg in ("v6", "v7", "v8", "v9"):
                nc.gpsimd.tensor_mul(m3[:], hi[:], rr[:])
            else:
                nc.vector.tensor_mul(m3[:], hi[:], rr[:])
            m4 = prod.tile([P, D], f32, tag="m4",
                           bufs=2 if cfg == "v9" else 1)
            if cfg == "v9":
                nc.gpsimd.tensor_mul(m4[:], hr[:], ri[:])
            else:
                nc.vector.tensor_mul(m4[:], hr[:], ri[:])
            m1 = prod.tile([P, D], f32, tag="m1")
            nc.vector.tensor_mul(m1[:], hr[:], rr[:])
            m2 = prod.tile([P, D], f32, tag="m2")
            nc.vector.tensor_mul(m2[:], hi[:], ri[:])
            ub = 1 if cfg == "v5" else 2
            ur = upool.tile([P, D], f32, tag="ur", bufs=ub)
            nc.vector.tensor_sub(ur[:], m1[:], m2[:])
            ui = upool.tile([P, D], f32, tag="ui", bufs=ub)
            if cfg == "v7" and t % 2 == 0:
                nc.gpsimd.tensor_add(ui[:], m3[:], m4[:])
            else:
                nc.vector.tensor_add(ui[:], m3[:], m4[:])

            # ab = sum(ur*tr) - sum(ui*ti): fused multiply+reduce via
            # scalar_tensor_tensor (out = (in0 op0 scalar) op1 in1,
            # accum_out = sum(out)). tensor_tensor_reduce (native TTR
            # opcode) crashes this terminal's NRT — do not use it.
            # Scratch outs alias the dead m1/m2 slots (same pool tag) —
            # WAR/WAW stay on-engine, zero extra SBUF.
            so1 = prod.tile([P, D], f32, tag="m1")
            nc.vector.scalar_tensor_tensor(
                out=so1[:], in0=ur[:], scalar=1.0, in1=tr[:],
                op0=Alu.mult, op1=Alu.mult, accum_out=ab1_s[:, t:t + 1],
            )
            so2 = prod.tile([P, D], f32, tag="m2")
            nc.vector.scalar_tensor_tensor(
                out=so2[:], in0=ui[:], scalar=-1.0, in1=ti[:],
                op0=Alu.mult, op1=Alu.mult, accum_out=ab2_s[:, t:t + 1],
            )

            # aa, bb: square+accumulate on ACT. bb first — tr/ti are
            # already resident before DVE finishes the products, so ACT
            # starts early and tr/ti stay hot for the STT dots.
            for src, dst in (
                (tr, bb1_s), (ti, bb2_s), (ur, aa1_s), (ui, aa2_s),
            ):
                ao = scr.tile([P, D], f32, tag="ao")
                nc.scalar.activation(
                    out=ao[:], in_=src[:], func=Act.Square,
                    accum_out=dst[:, t:t + 1],
                )

        # Final combine on [P, NT] (tiny).
        fin = {}
        def ftile(name):
            tl = stats.tile([P, NT], f32, tag=name)
            fin[name] = tl
            return tl

        ab = ftile("ab"); nc.vector.tensor_add(ab[:], ab1_s[:], ab2_s[:])
        aa = ftile("aa"); nc.vector.tensor_add(aa[:], aa1_s[:], aa2_s[:])
        bb = ftile("bb"); nc.vector.tensor_add(bb[:], bb1_s[:], bb2_s[:])
        pp = ftile("pp"); nc.vector.tensor_mul(pp[:], aa[:], bb[:])
        # sqrt on ACT is low precision (up to ~65536 ULP budget); refine
        # with two Newton iterations  r <- 0.5*(r + p/r)  using the
        # bit-exact DVE reciprocal.
        r = ftile("r0"); nc.scalar.activation(out=r[:], in_=pp[:], func=Act.Sqrt)
        for it in range(2):
            q = ftile(f"q{it}"); nc.vector.reciprocal(q[:], r[:])
            pq = ftile(f"pq{it}"); nc.vector.tensor_mul(pq[:], pp[:], q[:])
            s = ftile(f"s{it}"); nc.vector.tensor_add(s[:], r[:], pq[:])
            r = ftile(f"r{it + 1}"); nc.vector.tensor_scalar_mul(r[:], s[:], 0.5)
        inv = ftile("inv"); nc.vector.reciprocal(inv[:], r[:])
        score = ftile("score"); nc.vector.tensor_mul(score[:], ab[:], inv[:])
        nc.sync.dma_start(out=out_d, in_=score[:])


def _build(repeats=1, cfg="v11"):
    key = ("nc", repeats, cfg)
    if key in _CACHE:
        return _CACHE[key]
    import concourse.tile as tile
    from concourse import bacc, mybir

    # NOTE: num_devices is deliberately NOT set — it enables collective
    # global-comm setup that breaks plain SPMD input binding under the
    # axon/PJRT path (outputs come back as garbage).
    nc = bacc.Bacc(
        "TRN2",
        target_bir_lowering=False,
        debug=False,
    )
    ins = {
        n: nc.dram_tensor(n, [BC, D], mybir.dt.float32, kind="ExternalInput").ap()
        for n in IN_NAMES
    }
    out = nc.dram_tensor("out", [BC], mybir.dt.float32, kind="ExternalOutput").ap()
    with tile.TileContext(nc) as tc:
        _emit(tc, ins, out, mybir, repeats=repeats, cfg=cfg)
    nc.compile()
    _CACHE[key] = nc
    return nc


def make_in_maps(inputs):
    """Per-core input dicts: batch-sharded slices, dtype per kernel build."""
    in_maps = []
    for c in range(NCORES):
        sl = slice(c * BC, (c + 1) * BC)
        in_maps.append(
            {n: np.ascontiguousarray(inputs[n][sl], dtype=np.float32)
             for n in IN_NAMES}
        )
    return in_maps


def run(inputs, trace=False, **kwargs):
    """Run on 8 cores; returns (full_output, BassKernelResults)."""
    from concourse.bass_utils import run_bass_kernel_spmd

    nc = _build()
    core_ids = list(range(NCORES))
    in_maps = make_in_maps(inputs)
    # The terminal occasionally reports the accelerator unrecoverable
    # (e.g. poisoned by an earlier crashed run); a fresh attempt after a
    # short wait triggers recovery.
    last_exc = None
    for attempt in range(4):
        try:
            res = run_bass_kernel_spmd(nc, in_maps, core_ids, trace=trace, **kwargs)
            break
        except Exception as e:  # noqa: BLE001
            last_exc = e
            if attempt == 3:
                raise
            import time as _time
            _time.sleep(15 * (attempt + 1))
    out = np.concatenate([res.results[c]["out"] for c in range(NCORES)])
    return out.astype(np.float32), res


def kernel(**inputs):
    out, _ = run(inputs)
    return out

